# revision 40
# baseline (speedup 1.0000x reference)
"""DynamicLoRAConv1d kernel for 8 Trainium2 NeuronCores (~119 us HW).

Math: the per-sample LoRA conv is linear in weights, so
  conv(x, W) + conv(x, dW_b) = conv(x, W + dW_b)
with dW_b = lora_scale * (B_b @ A_b).  The tiny per-sample effective weight
(conv_w + dW_b) is fused on host.  Host prep also deinterleaves the padded
input on the time axis (even positions -> partitions 0..63, odd -> 64..127,
bf16, image-inner DRAM layout), so conv tap pairs (2m, 2m+1) fuse into
K=128 unit-stride matmuls: 3 matmuls per 512-column half (taps (0,1),
(2,3) at K=128, tap 4 at K=64) accumulated in PSUM.

Per image, software-pipelined (stats skewed 2 images, final pass 3):
  stage_a: DMA-in, 6 bf16 matmuls, bias+ReLU on ScalarE (PSUM -> bf16 y),
           per-channel mean/var via DVE bn_stats.
  stage_b: GroupNorm(4 groups x 32ch) group reduce+broadcast across
           partitions with two DVE 32x32 block transposes (transpose ->
           free-dim reduce -> broadcast-scale -> transpose back) - no PE,
           no PSUM; then a short (128,1) chain on GpSimd/ScalarE/DVE
           produces per-channel scale/offset.
  stage_c: out = y*scale+offset, split DVE/GpSimd, fp16 out tile, DMA out.
Output is fp16 on device (post-GroupNorm range ~ +-9, quantization error
~4e-3 absolute) and upcast to fp32 on host.

Sharding: data-parallel over Batch - core c gets samples 4c..4c+3
(= images 32c..32c+32).  No cross-core communication.
"""

import sys
from contextlib import ExitStack

import numpy as np

for _p in ("/opt/trn_rl_repo", "/opt/pypackages"):
    if _p not in sys.path:
        sys.path.append(_p)

import concourse.bacc as bacc
import concourse.bass as bass
import concourse.mybir as mybir
import concourse.tile as tile
from concourse.bass_utils import run_bass_kernel_spmd

F32 = mybir.dt.float32
F32R = mybir.dt.float32r
BF16 = mybir.dt.bfloat16
FP16 = mybir.dt.float16
import os as _os
IN_DT = BF16 if _os.environ.get("KERNEL_IN_DT", "bf16") == "bf16" else F32R
AF = mybir.ActivationFunctionType
ALU = mybir.AluOpType

N_CORES = 8
SAMPLES = 4      # samples per core
SENSORS = 8
IMGS = SAMPLES * SENSORS  # images per core
IN_C = 64
OUT_C = 128
KTAPS = 5
T = 2048
T_PAD = T + 4    # 2052
T_HALF = T_PAD // 2  # 1026 deinterleaved columns
T_OUT = 1024
HALF = 512
EPS = 1e-5
G = 4
CPG = OUT_C // G  # channels per group
CPG_F = 32       # DVE block-transpose size (== CPG)

# Knobs for experiments (run_bass_kernel_spmd kwargs threaded by caller)
TRACE = False
LAST_RESULTS = None

_PROGRAM = None


def _build_program():
    nc = bacc.Bacc("TRN2", target_bir_lowering=False, debug=False)
    xin = nc.dram_tensor("xin", [2 * IN_C, IMGS, T_HALF], IN_DT, kind="ExternalInput")
    wts = nc.dram_tensor("wts", [SAMPLES, 2 * IN_C, 3 * OUT_C], IN_DT,
                         kind="ExternalInput")
    cons = nc.dram_tensor("cons", [OUT_C, 4], F32, kind="ExternalInput")
    out = nc.dram_tensor("out", [OUT_C, IMGS, T_OUT], FP16, kind="ExternalOutput")

    with ExitStack() as ctx:
        tc = ctx.enter_context(tile.TileContext(nc))
        cpool = ctx.enter_context(tc.tile_pool(name="cpool", bufs=1))
        xpool = ctx.enter_context(tc.tile_pool(name="xpool", bufs=12))
        ypool = ctx.enter_context(tc.tile_pool(name="ypool", bufs=3))
        opool = ctx.enter_context(tc.tile_pool(name="opool", bufs=3))
        spool = ctx.enter_context(tc.tile_pool(name="spool", bufs=4))
        pspool = ctx.enter_context(tc.tile_pool(name="pspool", bufs=4, space="PSUM"))

        # ---- persistent constants ----
        wt = cpool.tile([2 * IN_C, SAMPLES * 3 * OUT_C], IN_DT)
        for s in range(SAMPLES):
            nc.sync.dma_start(
                out=wt[:, s * 3 * OUT_C:(s + 1) * 3 * OUT_C],
                in_=wts.ap()[s])
        ct = cpool.tile([OUT_C, 4], F32)
        nc.sync.dma_start(out=ct[:], in_=cons.ap()[:])
        bias_ap = ct[:, 0:1]
        gamma_ap = ct[:, 1:2]
        beta_ap = ct[:, 2:3]
        eps_ap = ct[:, 3:4]
        # constant 1/CPG tile for the group-mean broadcast
        c32 = cpool.tile([OUT_C, CPG_F], F32)
        nc.gpsimd.memset(c32[:], 1.0 / CPG)

        state = {}

        def stage_a(i):
            """DMA-in, conv matmuls, bias+relu, bn_stats."""
            s = i // SENSORS
            # host-deinterleaved padded image:
            #   xt[ci, u]      = x_pad[ci, 2u]       (even positions)
            #   xt[64+ci, u]   = x_pad[ci, 2u+1]     (odd positions)
            # so column u = t+m supplies taps (2m, 2m+1) across 128
            # partitions -> tap pairs fuse into K=128 matmuls, unit stride.
            xt = xpool.tile([2 * IN_C, T_HALF], IN_DT, tag="xt",
                            name=f"xt_{i}")
            nc.sync.dma_start(out=xt[:], in_=xin.ap()[:, i, :])

            y = ypool.tile([OUT_C, T_OUT], BF16, tag="y", name=f"y_{i}")
            bnraw = spool.tile([OUT_C, 12], F32, tag="bnraw", name=f"bnraw_{i}")
            ps = [pspool.tile([OUT_C, HALF], F32, tag=f"ps{h}", name=f"ps{h}_{i}")
                  for h in range(2)]

            # conv: out[co, t] = sum_{k, ci} W[co,ci,k] * x_pad[ci, 2t+k]
            # tap pairs (0,1), (2,3) as K=128 matmuls; tap 4 as K=64
            for m in range(3):
                w_ap = wt[0:(2 * IN_C if m < 2 else IN_C),
                          (s * 3 + m) * OUT_C:(s * 3 + m + 1) * OUT_C]
                for h in range(2):
                    u0 = m + h * HALF
                    rhs = xt[0:(2 * IN_C if m < 2 else IN_C), u0:u0 + HALF]
                    nc.tensor.matmul(ps[h][:], w_ap, rhs,
                                     start=(m == 0), stop=(m == 2))

            # bias + relu, then per-channel mean/var via bn_stats
            for h in range(2):
                yh = y[:, h * HALF:(h + 1) * HALF]
                nc.scalar.activation(yh, ps[h][:], AF.Relu,
                                     bias=bias_ap, scale=1.0)
                nc.vector.bn_stats(bnraw[:, 6 * h:6 * h + 6], yh)
            state[i] = {"y": y, "bnraw": bnraw}

        def stage_b(i):
            """Group stats -> per-channel scale/offset (DVE+GpSimd+ACT, no PE).

            Cross-partition group reduce+broadcast via two DVE 32x32 block
            transposes: rows {32g, 32g+1} of the transposed tile hold the
            group's per-channel [mean_p, E2_p] along the free dim; a free-dim
            reduce then a broadcast-scale and a transpose back yield
            per-channel [mean_g, E2_g] with no PE involvement.
            """
            sti = state[i]
            # sq2 cols 0:2 = [mean_p, E2_p]  (E2 = var + mean^2); rest zero
            sq2 = spool.tile([OUT_C, CPG_F], F32, tag="sq2", name=f"sq2_{i}")
            tmp0 = spool.tile([OUT_C, 1], F32, tag="tmp0", name=f"tmp0_{i}")
            nc.gpsimd.memset(sq2[:, 2:CPG_F], 0.0)
            nc.vector.bn_aggr(sq2[:, 0:2], sti["bnraw"][:])
            nc.gpsimd.tensor_mul(tmp0[:], sq2[:, 0:1], sq2[:, 0:1])
            nc.gpsimd.tensor_add(sq2[:, 1:2], sq2[:, 1:2], tmp0[:])

            tr = spool.tile([OUT_C, CPG_F], F32, tag="tr", name=f"tr_{i}")
            nc.vector.transpose(tr[:], sq2[:])
            red = spool.tile([OUT_C, 1], F32, tag="red", name=f"red_{i}")
            nc.vector.reduce_sum(red[:], tr[:], axis=mybir.AxisListType.X)
            bc = spool.tile([OUT_C, CPG_F], F32, tag="bc", name=f"bc_{i}")
            nc.vector.tensor_scalar_mul(bc[:], c32[:], red[:])
            tr2 = spool.tile([OUT_C, CPG_F], F32, tag="tr2", name=f"tr2_{i}")
            nc.vector.transpose(tr2[:], bc[:])
            mean_g = tr2[:, 0:1]
            e2_g = tr2[:, 1:2]

            # per-channel scale/offset from group stats
            stat = spool.tile([OUT_C, 5], F32, tag="stat", name=f"stat_{i}")
            m2, var, std, rstd, tmp = (stat[:, j:j + 1] for j in range(5))
            nc.gpsimd.tensor_mul(m2, mean_g, mean_g)
            nc.gpsimd.tensor_sub(var, e2_g, m2)
            nc.scalar.activation(std, var, AF.Sqrt, bias=eps_ap)
            nc.vector.reciprocal(rstd, std)
            so = spool.tile([OUT_C, 2], F32, tag="so", name=f"so_{i}")
            scl = so[:, 0:1]
            off = so[:, 1:2]
            nc.gpsimd.tensor_mul(scl, rstd, gamma_ap)
            nc.gpsimd.tensor_mul(tmp, mean_g, scl)
            nc.gpsimd.tensor_sub(off, beta_ap, tmp)
            sti["so"] = so

        def stage_c(i):
            """Final y*scale+offset: half on ACT, half on GpSimd; DMA out."""
            sti = state.pop(i)
            so = sti["so"]
            scl = so[:, 0:1]
            off = so[:, 1:2]
            y = sti["y"]
            ot = opool.tile([OUT_C, T_OUT], FP16, tag="ot", name=f"ot_{i}")
            nc.vector.tensor_scalar(ot[:, 0:HALF], y[:, 0:HALF],
                                    scl, off, op0=ALU.mult, op1=ALU.add)
            nc.gpsimd.tensor_scalar(ot[:, HALF:T_OUT], y[:, HALF:T_OUT],
                                    scl, off, op0=ALU.mult, op1=ALU.add)
            nc.scalar.dma_start(out=out.ap()[:, i, :], in_=ot[:])

        for i in range(IMGS + 3):
            if i < IMGS:
                stage_a(i)
            if 2 <= i < IMGS + 2:
                stage_b(i - 2)
            if i >= 3:
                stage_c(i - 3)
    nc.compile()
    return nc


def get_program():
    global _PROGRAM
    if _PROGRAM is None:
        _PROGRAM = _build_program()
    return _PROGRAM


def _host_prep(x, A_flat, B_flat, conv_w, conv_b, gamma, beta, num_sensors, r,
               lora_scale):
    x = np.asarray(x, dtype=np.float32)
    A_flat = np.asarray(A_flat, dtype=np.float32)
    B_flat = np.asarray(B_flat, dtype=np.float32)
    conv_w = np.asarray(conv_w, dtype=np.float32)
    conv_b = np.asarray(conv_b, dtype=np.float32)
    gamma = np.asarray(gamma, dtype=np.float32)
    beta = np.asarray(beta, dtype=np.float32)
    batch = A_flat.shape[0]
    out_c, in_c, k = conv_w.shape
    ns = int(num_sensors)
    rr = int(r)
    ls = float(lora_scale)
    assert (batch, out_c, in_c, k) == (32, OUT_C, IN_C, KTAPS)
    assert ns == SENSORS and x.shape == (batch * ns, in_c, T)

    # per-sample effective weight, transposed for the PE (lhsT layout)
    A = A_flat.reshape(batch, rr, in_c * k)
    Bm = B_flat.reshape(batch, out_c, rr)
    delta = np.einsum("bor,brm->bom", Bm, A) * ls
    W = conv_w.reshape(1, out_c, in_c * k) + delta            # (B, out_c, in_c*k)
    WT = W.reshape(batch, out_c, in_c, k).transpose(0, 2, 3, 1)  # (B, ci, k, co)
    # pack tap pairs on the partition axis: tile m rows = [W_T[:, 2m], W_T[:, 2m+1]]
    Wt = np.zeros((batch, 2 * in_c, 3 * out_c), dtype=np.float32)
    for m in range(3):
        Wt[:, 0:in_c, m * out_c:(m + 1) * out_c] = WT[:, :, 2 * m, :]
        if 2 * m + 1 < k:
            Wt[:, in_c:2 * in_c, m * out_c:(m + 1) * out_c] = WT[:, :, 2 * m + 1, :]

    import ml_dtypes
    np_in_dt = (ml_dtypes.bfloat16 if IN_DT == BF16 else np.float32)
    # deinterleaved, padded, image-inner: [ci, n, u] = x_pad[n, ci, 2u];
    # [64+ci, n, u] = x_pad[n, ci, 2u+1]
    x_pad = np.zeros((2 * in_c, batch * ns, T_HALF), dtype=np_in_dt)
    x_pad[0:in_c, :, 1:1 + T // 2] = x[:, :, 0::2].transpose(1, 0, 2)
    x_pad[in_c:2 * in_c, :, 1:1 + T // 2] = x[:, :, 1::2].transpose(1, 0, 2)

    eps_col = np.full_like(conv_b, EPS)
    cons = np.ascontiguousarray(np.stack([conv_b, gamma, beta, eps_col], axis=1),
                                dtype=np.float32)
    in_maps = []
    for c in range(N_CORES):
        in_maps.append({
            "xin": np.ascontiguousarray(x_pad[:, c * IMGS:(c + 1) * IMGS]),
            "wts": np.ascontiguousarray(Wt[c * SAMPLES:(c + 1) * SAMPLES],
                                        dtype=np_in_dt),
            "cons": cons,
        })
    return in_maps


def _maybe_reset_devices():
    """Best-effort NRT reset (recovers a wedged core from a prior crash)."""
    try:
        import ctypes
        lib = ctypes.CDLL("/opt/axon/libaxon_pjrt.so")
        lib.axon_reset.restype = ctypes.c_int64
        lib.axon_reset()
    except Exception:
        pass


def kernel(x, A_flat, B_flat, conv_w, conv_b, gamma, beta, num_sensors, r,
           lora_scale):
    global LAST_RESULTS
    _maybe_reset_devices()
    in_maps = _host_prep(x, A_flat, B_flat, conv_w, conv_b, gamma, beta,
                         num_sensors, r, lora_scale)
    nc = get_program()
    res = run_bass_kernel_spmd(nc, in_maps, core_ids=list(range(N_CORES)),
                               trace=TRACE)
    LAST_RESULTS = res
    full = np.concatenate([res.results[c]["out"] for c in range(N_CORES)],
                          axis=1)                      # (OUT_C, 256, T_OUT)
    return np.ascontiguousarray(full.transpose(1, 0, 2), dtype=np.float32)


# revision 41
# speedup vs baseline: 1.1839x; 1.1839x over previous
"""DynamicLoRAConv1d kernel for 8 Trainium2 NeuronCores (~119 us HW).

Math: the per-sample LoRA conv is linear in weights, so
  conv(x, W) + conv(x, dW_b) = conv(x, W + dW_b)
with dW_b = lora_scale * (B_b @ A_b).  The tiny per-sample effective weight
(conv_w + dW_b) is fused on host.  Host prep also deinterleaves the padded
input on the time axis (even positions -> partitions 0..63, odd -> 64..127,
bf16, image-inner DRAM layout), so conv tap pairs (2m, 2m+1) fuse into
K=128 unit-stride matmuls: 3 matmuls per 512-column half (taps (0,1),
(2,3) at K=128, tap 4 at K=64) accumulated in PSUM.

Per image, software-pipelined (stats skewed 2 images, final pass 3):
  stage_a: DMA-in, 6 bf16 matmuls, bias+ReLU on ScalarE (PSUM -> bf16 y),
           per-channel mean/var via DVE bn_stats.
  stage_b: GroupNorm(4 groups x 32ch) group reduce+broadcast across
           partitions with two DVE 32x32 block transposes (transpose ->
           free-dim reduce -> broadcast-scale -> transpose back) - no PE,
           no PSUM; then a short (128,1) chain on GpSimd/ScalarE/DVE
           produces per-channel scale/offset.
  stage_c: out = y*scale+offset, split DVE/GpSimd, fp16 out tile, DMA out.
Output is fp16 on device (post-GroupNorm range ~ +-9, quantization error
~4e-3 absolute) and upcast to fp32 on host.

Sharding: data-parallel over Batch - core c gets samples 4c..4c+3
(= images 32c..32c+32).  No cross-core communication.
"""

import sys
from contextlib import ExitStack

import numpy as np

for _p in ("/opt/trn_rl_repo", "/opt/pypackages"):
    if _p not in sys.path:
        sys.path.append(_p)

import concourse.bacc as bacc
import concourse.bass as bass
import concourse.mybir as mybir
import concourse.tile as tile
from concourse.bass_utils import run_bass_kernel_spmd

F32 = mybir.dt.float32
F32R = mybir.dt.float32r
BF16 = mybir.dt.bfloat16
FP16 = mybir.dt.float16
import os as _os
IN_DT = BF16 if _os.environ.get("KERNEL_IN_DT", "bf16") == "bf16" else F32R
AF = mybir.ActivationFunctionType
ALU = mybir.AluOpType

N_CORES = 8
SAMPLES = 4      # samples per core
SENSORS = 8
IMGS = SAMPLES * SENSORS  # images per core
IN_C = 64
OUT_C = 128
KTAPS = 5
T = 2048
T_PAD = T + 4    # 2052
T_HALF = T_PAD // 2  # 1026 deinterleaved columns
T_OUT = 1024
HALF = 512
EPS = 1e-5
G = 4
CPG = OUT_C // G  # channels per group
CPG_F = 32       # DVE block-transpose size (== CPG)

# Knobs for experiments (run_bass_kernel_spmd kwargs threaded by caller)
TRACE = False
LAST_RESULTS = None

_PROGRAM = None


def _build_program():
    nc = bacc.Bacc("TRN2", target_bir_lowering=False, debug=False)
    xin = nc.dram_tensor("xin", [2 * IN_C, IMGS, T_HALF], IN_DT, kind="ExternalInput")
    wts = nc.dram_tensor("wts", [SAMPLES, 2 * IN_C, 3 * OUT_C], IN_DT,
                         kind="ExternalInput")
    cons = nc.dram_tensor("cons", [OUT_C, 4], F32, kind="ExternalInput")
    out = nc.dram_tensor("out", [OUT_C, IMGS, T_OUT], FP16, kind="ExternalOutput")

    with ExitStack() as ctx:
        tc = ctx.enter_context(tile.TileContext(nc))
        cpool = ctx.enter_context(tc.tile_pool(name="cpool", bufs=1))
        xpool = ctx.enter_context(tc.tile_pool(name="xpool", bufs=12))
        ypool = ctx.enter_context(tc.tile_pool(name="ypool", bufs=5))
        opool = ctx.enter_context(tc.tile_pool(name="opool", bufs=3))
        spool = ctx.enter_context(tc.tile_pool(name="spool", bufs=4))
        pspool = ctx.enter_context(tc.tile_pool(name="pspool", bufs=4, space="PSUM"))

        # ---- persistent constants ----
        wt = cpool.tile([2 * IN_C, SAMPLES * 3 * OUT_C], IN_DT)
        for s in range(SAMPLES):
            nc.sync.dma_start(
                out=wt[:, s * 3 * OUT_C:(s + 1) * 3 * OUT_C],
                in_=wts.ap()[s])
        ct = cpool.tile([OUT_C, 4], F32)
        nc.sync.dma_start(out=ct[:], in_=cons.ap()[:])
        bias_ap = ct[:, 0:1]
        gamma_ap = ct[:, 1:2]
        beta_ap = ct[:, 2:3]
        eps_ap = ct[:, 3:4]
        # constant 1/CPG tile for the group-mean broadcast
        c32 = cpool.tile([OUT_C, CPG_F], F32)
        nc.gpsimd.memset(c32[:], 1.0 / CPG)

        state = {}

        def stage_a(i):
            """DMA-in, conv matmuls, bias+relu, bn_stats."""
            s = i // SENSORS
            # host-deinterleaved padded image:
            #   xt[ci, u]      = x_pad[ci, 2u]       (even positions)
            #   xt[64+ci, u]   = x_pad[ci, 2u+1]     (odd positions)
            # so column u = t+m supplies taps (2m, 2m+1) across 128
            # partitions -> tap pairs fuse into K=128 matmuls, unit stride.
            xt = xpool.tile([2 * IN_C, T_HALF], IN_DT, tag="xt",
                            name=f"xt_{i}")
            nc.sync.dma_start(out=xt[:], in_=xin.ap()[:, i, :])

            y = ypool.tile([OUT_C, T_OUT], BF16, tag="y", name=f"y_{i}")
            bnraw = spool.tile([OUT_C, 12], F32, tag="bnraw", name=f"bnraw_{i}")
            ps = [pspool.tile([OUT_C, HALF], F32, tag=f"ps{h}", name=f"ps{h}_{i}")
                  for h in range(2)]

            # conv: out[co, t] = sum_{k, ci} W[co,ci,k] * x_pad[ci, 2t+k]
            # tap pairs (0,1), (2,3) as K=128 matmuls; tap 4 as K=64
            for m in range(3):
                w_ap = wt[0:(2 * IN_C if m < 2 else IN_C),
                          (s * 3 + m) * OUT_C:(s * 3 + m + 1) * OUT_C]
                for h in range(2):
                    u0 = m + h * HALF
                    rhs = xt[0:(2 * IN_C if m < 2 else IN_C), u0:u0 + HALF]
                    nc.tensor.matmul(ps[h][:], w_ap, rhs,
                                     start=(m == 0), stop=(m == 2))

            # bias + relu, then per-channel mean/var via bn_stats
            for h in range(2):
                yh = y[:, h * HALF:(h + 1) * HALF]
                nc.scalar.activation(yh, ps[h][:], AF.Relu,
                                     bias=bias_ap, scale=1.0)
                nc.vector.bn_stats(bnraw[:, 6 * h:6 * h + 6], yh)
            state[i] = {"y": y, "bnraw": bnraw}

        def stage_b(i):
            """Group stats -> per-channel scale/offset (DVE+GpSimd+ACT, no PE).

            Cross-partition group reduce+broadcast via two DVE 32x32 block
            transposes: rows {32g, 32g+1} of the transposed tile hold the
            group's per-channel [mean_p, E2_p] along the free dim; a free-dim
            reduce then a broadcast-scale and a transpose back yield
            per-channel [mean_g, E2_g] with no PE involvement.
            """
            sti = state[i]
            # sq2 cols 0:2 = [mean_p, E2_p]  (E2 = var + mean^2); rest zero
            sq2 = spool.tile([OUT_C, CPG_F], F32, tag="sq2", name=f"sq2_{i}")
            tmp0 = spool.tile([OUT_C, 1], F32, tag="tmp0", name=f"tmp0_{i}")
            nc.gpsimd.memset(sq2[:, 2:CPG_F], 0.0)
            nc.vector.bn_aggr(sq2[:, 0:2], sti["bnraw"][:])
            nc.gpsimd.tensor_mul(tmp0[:], sq2[:, 0:1], sq2[:, 0:1])
            nc.gpsimd.tensor_add(sq2[:, 1:2], sq2[:, 1:2], tmp0[:])

            tr = spool.tile([OUT_C, CPG_F], F32, tag="tr", name=f"tr_{i}")
            nc.vector.transpose(tr[:], sq2[:])
            red = spool.tile([OUT_C, 1], F32, tag="red", name=f"red_{i}")
            nc.vector.reduce_sum(red[:], tr[:], axis=mybir.AxisListType.X)
            bc = spool.tile([OUT_C, CPG_F], F32, tag="bc", name=f"bc_{i}")
            nc.vector.tensor_scalar_mul(bc[:], c32[:], red[:])
            tr2 = spool.tile([OUT_C, CPG_F], F32, tag="tr2", name=f"tr2_{i}")
            nc.vector.transpose(tr2[:], bc[:])
            mean_g = tr2[:, 0:1]
            e2_g = tr2[:, 1:2]

            # per-channel scale/offset from group stats
            stat = spool.tile([OUT_C, 5], F32, tag="stat", name=f"stat_{i}")
            m2, var, std, rstd, tmp = (stat[:, j:j + 1] for j in range(5))
            nc.gpsimd.tensor_mul(m2, mean_g, mean_g)
            nc.gpsimd.tensor_sub(var, e2_g, m2)
            nc.scalar.activation(std, var, AF.Sqrt, bias=eps_ap)
            nc.vector.reciprocal(rstd, std)
            so = spool.tile([OUT_C, 2], F32, tag="so", name=f"so_{i}")
            scl = so[:, 0:1]
            off = so[:, 1:2]
            nc.gpsimd.tensor_mul(scl, rstd, gamma_ap)
            nc.gpsimd.tensor_mul(tmp, mean_g, scl)
            nc.gpsimd.tensor_sub(off, beta_ap, tmp)
            sti["so"] = so

        def stage_c(i):
            """Final y*scale+offset: half on ACT, half on GpSimd; DMA out."""
            sti = state.pop(i)
            so = sti["so"]
            scl = so[:, 0:1]
            off = so[:, 1:2]
            y = sti["y"]
            ot = opool.tile([OUT_C, T_OUT], FP16, tag="ot", name=f"ot_{i}")
            nc.vector.tensor_scalar(ot[:, 0:HALF], y[:, 0:HALF],
                                    scl, off, op0=ALU.mult, op1=ALU.add)
            nc.gpsimd.tensor_scalar(ot[:, HALF:T_OUT], y[:, HALF:T_OUT],
                                    scl, off, op0=ALU.mult, op1=ALU.add)
            nc.scalar.dma_start(out=out.ap()[:, i, :], in_=ot[:])

        for i in range(IMGS + 3):
            if i < IMGS:
                stage_a(i)
            if 2 <= i < IMGS + 2:
                stage_b(i - 2)
            if i >= 3:
                stage_c(i - 3)
    nc.compile()
    return nc


def get_program():
    global _PROGRAM
    if _PROGRAM is None:
        _PROGRAM = _build_program()
    return _PROGRAM


def _host_prep(x, A_flat, B_flat, conv_w, conv_b, gamma, beta, num_sensors, r,
               lora_scale):
    x = np.asarray(x, dtype=np.float32)
    A_flat = np.asarray(A_flat, dtype=np.float32)
    B_flat = np.asarray(B_flat, dtype=np.float32)
    conv_w = np.asarray(conv_w, dtype=np.float32)
    conv_b = np.asarray(conv_b, dtype=np.float32)
    gamma = np.asarray(gamma, dtype=np.float32)
    beta = np.asarray(beta, dtype=np.float32)
    batch = A_flat.shape[0]
    out_c, in_c, k = conv_w.shape
    ns = int(num_sensors)
    rr = int(r)
    ls = float(lora_scale)
    assert (batch, out_c, in_c, k) == (32, OUT_C, IN_C, KTAPS)
    assert ns == SENSORS and x.shape == (batch * ns, in_c, T)

    # per-sample effective weight, transposed for the PE (lhsT layout)
    A = A_flat.reshape(batch, rr, in_c * k)
    Bm = B_flat.reshape(batch, out_c, rr)
    delta = np.einsum("bor,brm->bom", Bm, A) * ls
    W = conv_w.reshape(1, out_c, in_c * k) + delta            # (B, out_c, in_c*k)
    WT = W.reshape(batch, out_c, in_c, k).transpose(0, 2, 3, 1)  # (B, ci, k, co)
    # pack tap pairs on the partition axis: tile m rows = [W_T[:, 2m], W_T[:, 2m+1]]
    Wt = np.zeros((batch, 2 * in_c, 3 * out_c), dtype=np.float32)
    for m in range(3):
        Wt[:, 0:in_c, m * out_c:(m + 1) * out_c] = WT[:, :, 2 * m, :]
        if 2 * m + 1 < k:
            Wt[:, in_c:2 * in_c, m * out_c:(m + 1) * out_c] = WT[:, :, 2 * m + 1, :]

    import ml_dtypes
    np_in_dt = (ml_dtypes.bfloat16 if IN_DT == BF16 else np.float32)
    # deinterleaved, padded, image-inner: [ci, n, u] = x_pad[n, ci, 2u];
    # [64+ci, n, u] = x_pad[n, ci, 2u+1]
    x_pad = np.zeros((2 * in_c, batch * ns, T_HALF), dtype=np_in_dt)
    x_pad[0:in_c, :, 1:1 + T // 2] = x[:, :, 0::2].transpose(1, 0, 2)
    x_pad[in_c:2 * in_c, :, 1:1 + T // 2] = x[:, :, 1::2].transpose(1, 0, 2)

    eps_col = np.full_like(conv_b, EPS)
    cons = np.ascontiguousarray(np.stack([conv_b, gamma, beta, eps_col], axis=1),
                                dtype=np.float32)
    in_maps = []
    for c in range(N_CORES):
        in_maps.append({
            "xin": np.ascontiguousarray(x_pad[:, c * IMGS:(c + 1) * IMGS]),
            "wts": np.ascontiguousarray(Wt[c * SAMPLES:(c + 1) * SAMPLES],
                                        dtype=np_in_dt),
            "cons": cons,
        })
    return in_maps


def _maybe_reset_devices():
    """Best-effort NRT reset (recovers a wedged core from a prior crash)."""
    try:
        import ctypes
        lib = ctypes.CDLL("/opt/axon/libaxon_pjrt.so")
        lib.axon_reset.restype = ctypes.c_int64
        lib.axon_reset()
    except Exception:
        pass


def kernel(x, A_flat, B_flat, conv_w, conv_b, gamma, beta, num_sensors, r,
           lora_scale):
    global LAST_RESULTS
    _maybe_reset_devices()
    in_maps = _host_prep(x, A_flat, B_flat, conv_w, conv_b, gamma, beta,
                         num_sensors, r, lora_scale)
    nc = get_program()
    res = run_bass_kernel_spmd(nc, in_maps, core_ids=list(range(N_CORES)),
                               trace=TRACE)
    LAST_RESULTS = res
    full = np.concatenate([res.results[c]["out"] for c in range(N_CORES)],
                          axis=1)                      # (OUT_C, 256, T_OUT)
    return np.ascontiguousarray(full.transpose(1, 0, 2), dtype=np.float32)


# revision 42
# speedup vs baseline: 1.1857x; 1.0016x over previous
"""DynamicLoRAConv1d kernel for 8 Trainium2 NeuronCores (~119 us HW).

Math: the per-sample LoRA conv is linear in weights, so
  conv(x, W) + conv(x, dW_b) = conv(x, W + dW_b)
with dW_b = lora_scale * (B_b @ A_b).  The tiny per-sample effective weight
(conv_w + dW_b) is fused on host.  Host prep also deinterleaves the padded
input on the time axis (even positions -> partitions 0..63, odd -> 64..127,
bf16, image-inner DRAM layout), so conv tap pairs (2m, 2m+1) fuse into
K=128 unit-stride matmuls: 3 matmuls per 512-column half (taps (0,1),
(2,3) at K=128, tap 4 at K=64) accumulated in PSUM.

Per image, software-pipelined (stats skewed 2 images, final pass 3):
  stage_a: DMA-in, 6 bf16 matmuls, bias+ReLU on ScalarE (PSUM -> bf16 y),
           per-channel mean/var via DVE bn_stats.
  stage_b: GroupNorm(4 groups x 32ch) group reduce+broadcast across
           partitions with two DVE 32x32 block transposes (transpose ->
           free-dim reduce -> broadcast-scale -> transpose back) - no PE,
           no PSUM; then a short (128,1) chain on GpSimd/ScalarE/DVE
           produces per-channel scale/offset.
  stage_c: out = y*scale+offset, split DVE/GpSimd, fp16 out tile, DMA out.
Output is fp16 on device (post-GroupNorm range ~ +-9, quantization error
~4e-3 absolute) and upcast to fp32 on host.

Sharding: data-parallel over Batch - core c gets samples 4c..4c+3
(= images 32c..32c+32).  No cross-core communication.
"""

import sys
from contextlib import ExitStack

import numpy as np

for _p in ("/opt/trn_rl_repo", "/opt/pypackages"):
    if _p not in sys.path:
        sys.path.append(_p)

import concourse.bacc as bacc
import concourse.bass as bass
import concourse.mybir as mybir
import concourse.tile as tile
from concourse.bass_utils import run_bass_kernel_spmd

F32 = mybir.dt.float32
F32R = mybir.dt.float32r
BF16 = mybir.dt.bfloat16
FP16 = mybir.dt.float16
import os as _os
IN_DT = BF16 if _os.environ.get("KERNEL_IN_DT", "bf16") == "bf16" else F32R
AF = mybir.ActivationFunctionType
ALU = mybir.AluOpType

N_CORES = 8
SAMPLES = 4      # samples per core
SENSORS = 8
IMGS = SAMPLES * SENSORS  # images per core
IN_C = 64
OUT_C = 128
KTAPS = 5
T = 2048
T_PAD = T + 4    # 2052
T_HALF = T_PAD // 2  # 1026 deinterleaved columns
T_OUT = 1024
HALF = 512
EPS = 1e-5
G = 4
CPG = OUT_C // G  # channels per group
CPG_F = 32       # DVE block-transpose size (== CPG)

# Knobs for experiments (run_bass_kernel_spmd kwargs threaded by caller)
TRACE = False
LAST_RESULTS = None

_PROGRAM = None


def _build_program():
    nc = bacc.Bacc("TRN2", target_bir_lowering=False, debug=False)
    xin = nc.dram_tensor("xin", [2 * IN_C, IMGS, T_HALF], IN_DT, kind="ExternalInput")
    wts = nc.dram_tensor("wts", [SAMPLES, 2 * IN_C, 3 * OUT_C], IN_DT,
                         kind="ExternalInput")
    cons = nc.dram_tensor("cons", [OUT_C, 4], F32, kind="ExternalInput")
    out = nc.dram_tensor("out", [OUT_C, IMGS, T_OUT], FP16, kind="ExternalOutput")

    with ExitStack() as ctx:
        tc = ctx.enter_context(tile.TileContext(nc))
        cpool = ctx.enter_context(tc.tile_pool(name="cpool", bufs=1))
        xpool = ctx.enter_context(tc.tile_pool(name="xpool", bufs=12))
        ypool = ctx.enter_context(tc.tile_pool(name="ypool", bufs=5))
        opool = ctx.enter_context(tc.tile_pool(name="opool", bufs=3))
        spool = ctx.enter_context(tc.tile_pool(name="spool", bufs=4))
        pspool = ctx.enter_context(tc.tile_pool(name="pspool", bufs=4, space="PSUM"))

        # ---- persistent constants ----
        wt = cpool.tile([2 * IN_C, SAMPLES * 3 * OUT_C], IN_DT)
        for s in range(SAMPLES):
            nc.sync.dma_start(
                out=wt[:, s * 3 * OUT_C:(s + 1) * 3 * OUT_C],
                in_=wts.ap()[s])
        ct = cpool.tile([OUT_C, 4], F32)
        nc.sync.dma_start(out=ct[:], in_=cons.ap()[:])
        bias_ap = ct[:, 0:1]
        gamma_ap = ct[:, 1:2]
        beta_ap = ct[:, 2:3]
        eps_ap = ct[:, 3:4]
        # constant 1/CPG tile for the group-mean broadcast
        c32 = cpool.tile([OUT_C, CPG_F], F32)
        nc.gpsimd.memset(c32[:], 1.0 / CPG)

        state = {}

        def stage_a(i):
            """DMA-in, conv matmuls, bias+relu, bn_stats."""
            s = i // SENSORS
            # host-deinterleaved padded image:
            #   xt[ci, u]      = x_pad[ci, 2u]       (even positions)
            #   xt[64+ci, u]   = x_pad[ci, 2u+1]     (odd positions)
            # so column u = t+m supplies taps (2m, 2m+1) across 128
            # partitions -> tap pairs fuse into K=128 matmuls, unit stride.
            xt = xpool.tile([2 * IN_C, T_HALF], IN_DT, tag="xt",
                            name=f"xt_{i}")
            nc.sync.dma_start(out=xt[:], in_=xin.ap()[:, i, :])

            y = ypool.tile([OUT_C, T_OUT], BF16, tag="y", name=f"y_{i}")
            bnraw = spool.tile([OUT_C, 12], F32, tag="bnraw", name=f"bnraw_{i}")
            ps = [pspool.tile([OUT_C, HALF], F32, tag=f"ps{h}", name=f"ps{h}_{i}")
                  for h in range(2)]

            # conv: out[co, t] = sum_{k, ci} W[co,ci,k] * x_pad[ci, 2t+k]
            # tap pairs (0,1), (2,3) as K=128 matmuls; tap 4 as K=64
            for m in range(3):
                w_ap = wt[0:(2 * IN_C if m < 2 else IN_C),
                          (s * 3 + m) * OUT_C:(s * 3 + m + 1) * OUT_C]
                for h in range(2):
                    u0 = m + h * HALF
                    rhs = xt[0:(2 * IN_C if m < 2 else IN_C), u0:u0 + HALF]
                    nc.tensor.matmul(ps[h][:], w_ap, rhs,
                                     start=(m == 0), stop=(m == 2))

            # bias + relu, then per-channel mean/var via bn_stats
            for h in range(2):
                yh = y[:, h * HALF:(h + 1) * HALF]
                nc.scalar.activation(yh, ps[h][:], AF.Relu,
                                     bias=bias_ap, scale=1.0)
                nc.vector.bn_stats(bnraw[:, 6 * h:6 * h + 6], yh)
            state[i] = {"y": y, "bnraw": bnraw}

        def stage_b(i):
            """Group stats -> per-channel scale/offset (DVE+GpSimd+ACT, no PE).

            Cross-partition group reduce+broadcast via two DVE 32x32 block
            transposes: rows {32g, 32g+1} of the transposed tile hold the
            group's per-channel [mean_p, E2_p] along the free dim; a free-dim
            reduce then a broadcast-scale and a transpose back yield
            per-channel [mean_g, E2_g] with no PE involvement.
            """
            sti = state[i]
            # sq2 cols 0:2 = [mean_p, E2_p]  (E2 = var + mean^2); rest zero
            sq2 = spool.tile([OUT_C, CPG_F], F32, tag="sq2", name=f"sq2_{i}")
            tmp0 = spool.tile([OUT_C, 1], F32, tag="tmp0", name=f"tmp0_{i}")
            nc.gpsimd.memset(sq2[:, 2:CPG_F], 0.0)
            nc.vector.bn_aggr(sq2[:, 0:2], sti["bnraw"][:])
            nc.gpsimd.tensor_mul(tmp0[:], sq2[:, 0:1], sq2[:, 0:1])
            nc.gpsimd.tensor_add(sq2[:, 1:2], sq2[:, 1:2], tmp0[:])

            tr = spool.tile([OUT_C, CPG_F], F32, tag="tr", name=f"tr_{i}")
            nc.vector.transpose(tr[:], sq2[:])
            red = spool.tile([OUT_C, 1], F32, tag="red", name=f"red_{i}")
            nc.vector.reduce_sum(red[:], tr[:], axis=mybir.AxisListType.X)
            bc = spool.tile([OUT_C, CPG_F], F32, tag="bc", name=f"bc_{i}")
            nc.vector.tensor_scalar_mul(bc[:], c32[:], red[:])
            tr2 = spool.tile([OUT_C, CPG_F], F32, tag="tr2", name=f"tr2_{i}")
            nc.vector.transpose(tr2[:], bc[:])
            mean_g = tr2[:, 0:1]
            e2_g = tr2[:, 1:2]

            # per-channel scale/offset from group stats
            stat = spool.tile([OUT_C, 5], F32, tag="stat", name=f"stat_{i}")
            m2, var, std, rstd, tmp = (stat[:, j:j + 1] for j in range(5))
            nc.gpsimd.tensor_mul(m2, mean_g, mean_g)
            nc.gpsimd.tensor_sub(var, e2_g, m2)
            nc.scalar.activation(std, var, AF.Sqrt, bias=eps_ap)
            nc.vector.reciprocal(rstd, std)
            so = spool.tile([OUT_C, 2], F32, tag="so", name=f"so_{i}")
            scl = so[:, 0:1]
            off = so[:, 1:2]
            nc.gpsimd.tensor_mul(scl, rstd, gamma_ap)
            nc.gpsimd.tensor_mul(tmp, mean_g, scl)
            nc.gpsimd.tensor_sub(off, beta_ap, tmp)
            sti["so"] = so

        def stage_c(i):
            """Final y*scale+offset: half on ACT, half on GpSimd; DMA out."""
            sti = state.pop(i)
            so = sti["so"]
            scl = so[:, 0:1]
            off = so[:, 1:2]
            y = sti["y"]
            ot = opool.tile([OUT_C, T_OUT], FP16, tag="ot", name=f"ot_{i}")
            nc.vector.tensor_scalar(ot[:, 0:HALF], y[:, 0:HALF],
                                    scl, off, op0=ALU.mult, op1=ALU.add)
            nc.gpsimd.tensor_scalar(ot[:, HALF:T_OUT], y[:, HALF:T_OUT],
                                    scl, off, op0=ALU.mult, op1=ALU.add)
            nc.scalar.dma_start(out=out.ap()[:, i, :], in_=ot[:])

        for i in range(IMGS + 4):
            if i < IMGS:
                stage_a(i)
            if 2 <= i < IMGS + 2:
                stage_b(i - 2)
            if i >= 4:
                stage_c(i - 4)
    nc.compile()
    return nc


def get_program():
    global _PROGRAM
    if _PROGRAM is None:
        _PROGRAM = _build_program()
    return _PROGRAM


def _host_prep(x, A_flat, B_flat, conv_w, conv_b, gamma, beta, num_sensors, r,
               lora_scale):
    x = np.asarray(x, dtype=np.float32)
    A_flat = np.asarray(A_flat, dtype=np.float32)
    B_flat = np.asarray(B_flat, dtype=np.float32)
    conv_w = np.asarray(conv_w, dtype=np.float32)
    conv_b = np.asarray(conv_b, dtype=np.float32)
    gamma = np.asarray(gamma, dtype=np.float32)
    beta = np.asarray(beta, dtype=np.float32)
    batch = A_flat.shape[0]
    out_c, in_c, k = conv_w.shape
    ns = int(num_sensors)
    rr = int(r)
    ls = float(lora_scale)
    assert (batch, out_c, in_c, k) == (32, OUT_C, IN_C, KTAPS)
    assert ns == SENSORS and x.shape == (batch * ns, in_c, T)

    # per-sample effective weight, transposed for the PE (lhsT layout)
    A = A_flat.reshape(batch, rr, in_c * k)
    Bm = B_flat.reshape(batch, out_c, rr)
    delta = np.einsum("bor,brm->bom", Bm, A) * ls
    W = conv_w.reshape(1, out_c, in_c * k) + delta            # (B, out_c, in_c*k)
    WT = W.reshape(batch, out_c, in_c, k).transpose(0, 2, 3, 1)  # (B, ci, k, co)
    # pack tap pairs on the partition axis: tile m rows = [W_T[:, 2m], W_T[:, 2m+1]]
    Wt = np.zeros((batch, 2 * in_c, 3 * out_c), dtype=np.float32)
    for m in range(3):
        Wt[:, 0:in_c, m * out_c:(m + 1) * out_c] = WT[:, :, 2 * m, :]
        if 2 * m + 1 < k:
            Wt[:, in_c:2 * in_c, m * out_c:(m + 1) * out_c] = WT[:, :, 2 * m + 1, :]

    import ml_dtypes
    np_in_dt = (ml_dtypes.bfloat16 if IN_DT == BF16 else np.float32)
    # deinterleaved, padded, image-inner: [ci, n, u] = x_pad[n, ci, 2u];
    # [64+ci, n, u] = x_pad[n, ci, 2u+1]
    x_pad = np.zeros((2 * in_c, batch * ns, T_HALF), dtype=np_in_dt)
    x_pad[0:in_c, :, 1:1 + T // 2] = x[:, :, 0::2].transpose(1, 0, 2)
    x_pad[in_c:2 * in_c, :, 1:1 + T // 2] = x[:, :, 1::2].transpose(1, 0, 2)

    eps_col = np.full_like(conv_b, EPS)
    cons = np.ascontiguousarray(np.stack([conv_b, gamma, beta, eps_col], axis=1),
                                dtype=np.float32)
    in_maps = []
    for c in range(N_CORES):
        in_maps.append({
            "xin": np.ascontiguousarray(x_pad[:, c * IMGS:(c + 1) * IMGS]),
            "wts": np.ascontiguousarray(Wt[c * SAMPLES:(c + 1) * SAMPLES],
                                        dtype=np_in_dt),
            "cons": cons,
        })
    return in_maps


def _maybe_reset_devices():
    """Best-effort NRT reset (recovers a wedged core from a prior crash)."""
    try:
        import ctypes
        lib = ctypes.CDLL("/opt/axon/libaxon_pjrt.so")
        lib.axon_reset.restype = ctypes.c_int64
        lib.axon_reset()
    except Exception:
        pass


def kernel(x, A_flat, B_flat, conv_w, conv_b, gamma, beta, num_sensors, r,
           lora_scale):
    global LAST_RESULTS
    _maybe_reset_devices()
    in_maps = _host_prep(x, A_flat, B_flat, conv_w, conv_b, gamma, beta,
                         num_sensors, r, lora_scale)
    nc = get_program()
    res = run_bass_kernel_spmd(nc, in_maps, core_ids=list(range(N_CORES)),
                               trace=TRACE)
    LAST_RESULTS = res
    full = np.concatenate([res.results[c]["out"] for c in range(N_CORES)],
                          axis=1)                      # (OUT_C, 256, T_OUT)
    return np.ascontiguousarray(full.transpose(1, 0, 2), dtype=np.float32)


# revision 43
# speedup vs baseline: 1.1924x; 1.0056x over previous
"""DynamicLoRAConv1d kernel for 8 Trainium2 NeuronCores (~119 us HW).

Math: the per-sample LoRA conv is linear in weights, so
  conv(x, W) + conv(x, dW_b) = conv(x, W + dW_b)
with dW_b = lora_scale * (B_b @ A_b).  The tiny per-sample effective weight
(conv_w + dW_b) is fused on host.  Host prep also deinterleaves the padded
input on the time axis (even positions -> partitions 0..63, odd -> 64..127,
bf16, image-inner DRAM layout), so conv tap pairs (2m, 2m+1) fuse into
K=128 unit-stride matmuls: 3 matmuls per 512-column half (taps (0,1),
(2,3) at K=128, tap 4 at K=64) accumulated in PSUM.

Per image, software-pipelined (stats skewed 2 images, final pass 3):
  stage_a: DMA-in, 6 bf16 matmuls, bias+ReLU on ScalarE (PSUM -> bf16 y),
           per-channel mean/var via DVE bn_stats.
  stage_b: GroupNorm(4 groups x 32ch) group reduce+broadcast across
           partitions with two DVE 32x32 block transposes (transpose ->
           free-dim reduce -> broadcast-scale -> transpose back) - no PE,
           no PSUM; then a short (128,1) chain on GpSimd/ScalarE/DVE
           produces per-channel scale/offset.
  stage_c: out = y*scale+offset, split DVE/GpSimd, fp16 out tile, DMA out.
Output is fp16 on device (post-GroupNorm range ~ +-9, quantization error
~4e-3 absolute) and upcast to fp32 on host.

Sharding: data-parallel over Batch - core c gets samples 4c..4c+3
(= images 32c..32c+32).  No cross-core communication.
"""

import sys
from contextlib import ExitStack

import numpy as np

for _p in ("/opt/trn_rl_repo", "/opt/pypackages"):
    if _p not in sys.path:
        sys.path.append(_p)

import concourse.bacc as bacc
import concourse.bass as bass
import concourse.mybir as mybir
import concourse.tile as tile
from concourse.bass_utils import run_bass_kernel_spmd

F32 = mybir.dt.float32
F32R = mybir.dt.float32r
BF16 = mybir.dt.bfloat16
FP16 = mybir.dt.float16
import os as _os
IN_DT = BF16 if _os.environ.get("KERNEL_IN_DT", "bf16") == "bf16" else F32R
AF = mybir.ActivationFunctionType
ALU = mybir.AluOpType

N_CORES = 8
SAMPLES = 4      # samples per core
SENSORS = 8
IMGS = SAMPLES * SENSORS  # images per core
IN_C = 64
OUT_C = 128
KTAPS = 5
T = 2048
T_PAD = T + 4    # 2052
T_HALF = T_PAD // 2  # 1026 deinterleaved columns
T_OUT = 1024
HALF = 512
EPS = 1e-5
G = 4
CPG = OUT_C // G  # channels per group
CPG_F = 32       # DVE block-transpose size (== CPG)

# Knobs for experiments (run_bass_kernel_spmd kwargs threaded by caller)
TRACE = False
LAST_RESULTS = None

_PROGRAM = None


def _build_program():
    nc = bacc.Bacc("TRN2", target_bir_lowering=False, debug=False)
    xin = nc.dram_tensor("xin", [2 * IN_C, IMGS, T_HALF], IN_DT, kind="ExternalInput")
    wts = nc.dram_tensor("wts", [SAMPLES, 2 * IN_C, 3 * OUT_C], IN_DT,
                         kind="ExternalInput")
    cons = nc.dram_tensor("cons", [OUT_C, 4], F32, kind="ExternalInput")
    out = nc.dram_tensor("out", [OUT_C, IMGS, T_OUT], FP16, kind="ExternalOutput")

    with ExitStack() as ctx:
        tc = ctx.enter_context(tile.TileContext(nc))
        cpool = ctx.enter_context(tc.tile_pool(name="cpool", bufs=1))
        xpool = ctx.enter_context(tc.tile_pool(name="xpool", bufs=12))
        ypool = ctx.enter_context(tc.tile_pool(name="ypool", bufs=5))
        opool = ctx.enter_context(tc.tile_pool(name="opool", bufs=3))
        spool = ctx.enter_context(tc.tile_pool(name="spool", bufs=4))
        pspool = ctx.enter_context(tc.tile_pool(name="pspool", bufs=4, space="PSUM"))

        # ---- persistent constants ----
        wt = cpool.tile([2 * IN_C, SAMPLES * 3 * OUT_C], IN_DT)
        for s in range(SAMPLES):
            nc.sync.dma_start(
                out=wt[:, s * 3 * OUT_C:(s + 1) * 3 * OUT_C],
                in_=wts.ap()[s])
        ct = cpool.tile([OUT_C, 4], F32)
        nc.sync.dma_start(out=ct[:], in_=cons.ap()[:])
        bias_ap = ct[:, 0:1]
        gamma_ap = ct[:, 1:2]
        beta_ap = ct[:, 2:3]
        eps_ap = ct[:, 3:4]
        # constant 1/CPG tile for the group-mean broadcast
        c32 = cpool.tile([OUT_C, CPG_F], F32)
        nc.gpsimd.memset(c32[:], 1.0 / CPG)

        state = {}

        def stage_a(i):
            """DMA-in, conv matmuls, bias+relu, bn_stats."""
            s = i // SENSORS
            # host-deinterleaved padded image:
            #   xt[ci, u]      = x_pad[ci, 2u]       (even positions)
            #   xt[64+ci, u]   = x_pad[ci, 2u+1]     (odd positions)
            # so column u = t+m supplies taps (2m, 2m+1) across 128
            # partitions -> tap pairs fuse into K=128 matmuls, unit stride.
            xt = xpool.tile([2 * IN_C, T_HALF], IN_DT, tag="xt",
                            name=f"xt_{i}")
            nc.sync.dma_start(out=xt[:], in_=xin.ap()[:, i, :])

            y = ypool.tile([OUT_C, T_OUT], BF16, tag="y", name=f"y_{i}")
            bnraw = spool.tile([OUT_C, 12], F32, tag="bnraw", name=f"bnraw_{i}")
            ps = [pspool.tile([OUT_C, HALF], F32, tag=f"ps{h}", name=f"ps{h}_{i}")
                  for h in range(2)]

            # conv: out[co, t] = sum_{k, ci} W[co,ci,k] * x_pad[ci, 2t+k]
            # tap pairs (0,1), (2,3) as K=128 matmuls; tap 4 as K=64
            for m in range(3):
                w_ap = wt[0:(2 * IN_C if m < 2 else IN_C),
                          (s * 3 + m) * OUT_C:(s * 3 + m + 1) * OUT_C]
                for h in range(2):
                    u0 = m + h * HALF
                    rhs = xt[0:(2 * IN_C if m < 2 else IN_C), u0:u0 + HALF]
                    nc.tensor.matmul(ps[h][:], w_ap, rhs,
                                     start=(m == 0), stop=(m == 2))

            # bias + relu, then per-channel mean/var via bn_stats
            for h in range(2):
                yh = y[:, h * HALF:(h + 1) * HALF]
                nc.scalar.activation(yh, ps[h][:], AF.Relu,
                                     bias=bias_ap, scale=1.0)
                nc.vector.bn_stats(bnraw[:, 6 * h:6 * h + 6], yh)
            state[i] = {"y": y, "bnraw": bnraw}

        def stage_b(i):
            """Group stats -> per-channel scale/offset (DVE+GpSimd+ACT, no PE).

            Cross-partition group reduce+broadcast via two DVE 32x32 block
            transposes: rows {32g, 32g+1} of the transposed tile hold the
            group's per-channel [mean_p, E2_p] along the free dim; a free-dim
            reduce then a broadcast-scale and a transpose back yield
            per-channel [mean_g, E2_g] with no PE involvement.
            """
            sti = state[i]
            # sq2 cols 0:2 = [mean_p, E2_p]  (E2 = var + mean^2); rest zero
            sq2 = spool.tile([OUT_C, CPG_F], F32, tag="sq2", name=f"sq2_{i}")
            tmp0 = spool.tile([OUT_C, 1], F32, tag="tmp0", name=f"tmp0_{i}")
            nc.gpsimd.memset(sq2[:, 2:CPG_F], 0.0)
            nc.vector.bn_aggr(sq2[:, 0:2], sti["bnraw"][:])
            nc.gpsimd.tensor_mul(tmp0[:], sq2[:, 0:1], sq2[:, 0:1])
            nc.gpsimd.tensor_add(sq2[:, 1:2], sq2[:, 1:2], tmp0[:])

            tr = spool.tile([OUT_C, CPG_F], F32, tag="tr", name=f"tr_{i}")
            nc.vector.transpose(tr[:], sq2[:])
            red = spool.tile([OUT_C, 1], F32, tag="red", name=f"red_{i}")
            nc.vector.reduce_sum(red[:], tr[:], axis=mybir.AxisListType.X)
            bc = spool.tile([OUT_C, CPG_F], F32, tag="bc", name=f"bc_{i}")
            nc.vector.tensor_scalar_mul(bc[:], c32[:], red[:])
            tr2 = spool.tile([OUT_C, CPG_F], F32, tag="tr2", name=f"tr2_{i}")
            nc.vector.transpose(tr2[:], bc[:])
            mean_g = tr2[:, 0:1]
            e2_g = tr2[:, 1:2]

            # per-channel scale/offset from group stats
            stat = spool.tile([OUT_C, 5], F32, tag="stat", name=f"stat_{i}")
            m2, var, std, rstd, tmp = (stat[:, j:j + 1] for j in range(5))
            nc.gpsimd.tensor_mul(m2, mean_g, mean_g)
            nc.gpsimd.tensor_sub(var, e2_g, m2)
            nc.scalar.activation(std, var, AF.Sqrt, bias=eps_ap)
            nc.vector.reciprocal(rstd, std)
            so = spool.tile([OUT_C, 2], F32, tag="so", name=f"so_{i}")
            scl = so[:, 0:1]
            off = so[:, 1:2]
            nc.gpsimd.tensor_mul(scl, rstd, gamma_ap)
            nc.gpsimd.tensor_mul(tmp, mean_g, scl)
            nc.gpsimd.tensor_sub(off, beta_ap, tmp)
            sti["so"] = so

        def stage_c(i):
            """Final y*scale+offset: half on ACT, half on GpSimd; DMA out."""
            sti = state.pop(i)
            so = sti["so"]
            scl = so[:, 0:1]
            off = so[:, 1:2]
            y = sti["y"]
            ot = opool.tile([OUT_C, T_OUT], FP16, tag="ot", name=f"ot_{i}")
            nc.vector.tensor_scalar(ot[:, 0:HALF], y[:, 0:HALF],
                                    scl, off, op0=ALU.mult, op1=ALU.add)
            nc.gpsimd.tensor_scalar(ot[:, HALF:T_OUT], y[:, HALF:T_OUT],
                                    scl, off, op0=ALU.mult, op1=ALU.add)
            nc.scalar.dma_start(out=out.ap()[:, i, :], in_=ot[:])

        for i in range(IMGS + 4):
            if i < IMGS:
                stage_a(i)
            if 3 <= i < IMGS + 3:
                stage_b(i - 3)
            if i >= 4:
                stage_c(i - 4)
    nc.compile()
    return nc


def get_program():
    global _PROGRAM
    if _PROGRAM is None:
        _PROGRAM = _build_program()
    return _PROGRAM


def _host_prep(x, A_flat, B_flat, conv_w, conv_b, gamma, beta, num_sensors, r,
               lora_scale):
    x = np.asarray(x, dtype=np.float32)
    A_flat = np.asarray(A_flat, dtype=np.float32)
    B_flat = np.asarray(B_flat, dtype=np.float32)
    conv_w = np.asarray(conv_w, dtype=np.float32)
    conv_b = np.asarray(conv_b, dtype=np.float32)
    gamma = np.asarray(gamma, dtype=np.float32)
    beta = np.asarray(beta, dtype=np.float32)
    batch = A_flat.shape[0]
    out_c, in_c, k = conv_w.shape
    ns = int(num_sensors)
    rr = int(r)
    ls = float(lora_scale)
    assert (batch, out_c, in_c, k) == (32, OUT_C, IN_C, KTAPS)
    assert ns == SENSORS and x.shape == (batch * ns, in_c, T)

    # per-sample effective weight, transposed for the PE (lhsT layout)
    A = A_flat.reshape(batch, rr, in_c * k)
    Bm = B_flat.reshape(batch, out_c, rr)
    delta = np.einsum("bor,brm->bom", Bm, A) * ls
    W = conv_w.reshape(1, out_c, in_c * k) + delta            # (B, out_c, in_c*k)
    WT = W.reshape(batch, out_c, in_c, k).transpose(0, 2, 3, 1)  # (B, ci, k, co)
    # pack tap pairs on the partition axis: tile m rows = [W_T[:, 2m], W_T[:, 2m+1]]
    Wt = np.zeros((batch, 2 * in_c, 3 * out_c), dtype=np.float32)
    for m in range(3):
        Wt[:, 0:in_c, m * out_c:(m + 1) * out_c] = WT[:, :, 2 * m, :]
        if 2 * m + 1 < k:
            Wt[:, in_c:2 * in_c, m * out_c:(m + 1) * out_c] = WT[:, :, 2 * m + 1, :]

    import ml_dtypes
    np_in_dt = (ml_dtypes.bfloat16 if IN_DT == BF16 else np.float32)
    # deinterleaved, padded, image-inner: [ci, n, u] = x_pad[n, ci, 2u];
    # [64+ci, n, u] = x_pad[n, ci, 2u+1]
    x_pad = np.zeros((2 * in_c, batch * ns, T_HALF), dtype=np_in_dt)
    x_pad[0:in_c, :, 1:1 + T // 2] = x[:, :, 0::2].transpose(1, 0, 2)
    x_pad[in_c:2 * in_c, :, 1:1 + T // 2] = x[:, :, 1::2].transpose(1, 0, 2)

    eps_col = np.full_like(conv_b, EPS)
    cons = np.ascontiguousarray(np.stack([conv_b, gamma, beta, eps_col], axis=1),
                                dtype=np.float32)
    in_maps = []
    for c in range(N_CORES):
        in_maps.append({
            "xin": np.ascontiguousarray(x_pad[:, c * IMGS:(c + 1) * IMGS]),
            "wts": np.ascontiguousarray(Wt[c * SAMPLES:(c + 1) * SAMPLES],
                                        dtype=np_in_dt),
            "cons": cons,
        })
    return in_maps


def _maybe_reset_devices():
    """Best-effort NRT reset (recovers a wedged core from a prior crash)."""
    try:
        import ctypes
        lib = ctypes.CDLL("/opt/axon/libaxon_pjrt.so")
        lib.axon_reset.restype = ctypes.c_int64
        lib.axon_reset()
    except Exception:
        pass


def kernel(x, A_flat, B_flat, conv_w, conv_b, gamma, beta, num_sensors, r,
           lora_scale):
    global LAST_RESULTS
    _maybe_reset_devices()
    in_maps = _host_prep(x, A_flat, B_flat, conv_w, conv_b, gamma, beta,
                         num_sensors, r, lora_scale)
    nc = get_program()
    res = run_bass_kernel_spmd(nc, in_maps, core_ids=list(range(N_CORES)),
                               trace=TRACE)
    LAST_RESULTS = res
    full = np.concatenate([res.results[c]["out"] for c in range(N_CORES)],
                          axis=1)                      # (OUT_C, 256, T_OUT)
    return np.ascontiguousarray(full.transpose(1, 0, 2), dtype=np.float32)
